# revision 1
# baseline (speedup 1.0000x reference)
"""Trainium2 Bass kernel for nn_CrossRQVAE (RQ-VAE forward pass).

Data-parallel over 8 NeuronCores: batch 32768 -> 4096 rows/core.
Per core, fully fused pipeline in fp32:
  encoder MLP (1024-512-256-128-32, transposed-activation layout)
  -> 3 residual VQ stages (fused distance matmul, argmin via max_index,
     one-hot gather via PE matmul)
  -> decoder MLP (32-128-256-512-1024, last layer flipped to produce
     natural-layout output directly).

Host side only shards/transposes x, packs weights, and concatenates
per-core outputs (plus summing the 8 scalar loss partials).
"""
import numpy as np

B, IN_DIM, E_DIM, N_E, N_Q = 32768, 1024, 32, 256, 3
ENC_DIMS = [1024, 512, 256, 128, 32]
DEC_DIMS = [32, 128, 256, 512, 1024]
BETA = 0.25
N_CORES = 8
BC = B // N_CORES          # rows per core
NB = 512                   # batch tile (free dim)
NT = BC // NB              # tiles per core
NSUB = NB // 128           # 128-row subtiles per tile

_CACHE = {}


def _build(dec_b3_nonzero: bool):
    import concourse.bacc as bacc
    import concourse.mybir as mybir
    import concourse.tile as tile

    F32 = mybir.dt.float32
    U32 = mybir.dt.uint32
    AF = mybir.ActivationFunctionType
    OP = mybir.AluOpType
    AX = mybir.AxisListType

    nc = bacc.Bacc()

    # ---------------- DRAM I/O ----------------
    xT_d = nc.dram_tensor("xT", [IN_DIM, BC], F32, kind="ExternalInput")
    wenc_d = [nc.dram_tensor(f"wenc{i}", [ENC_DIMS[i], ENC_DIMS[i + 1]], F32,
                             kind="ExternalInput") for i in range(4)]
    benc_d = [nc.dram_tensor(f"benc{i}",
                             [min(ENC_DIMS[i + 1], 128), max(ENC_DIMS[i + 1] // 128, 1)],
                             F32, kind="ExternalInput") for i in range(4)]
    wdec_d = [nc.dram_tensor(f"wdec{i}", [DEC_DIMS[i], DEC_DIMS[i + 1]], F32,
                             kind="ExternalInput") for i in range(4)]
    bdec_d = [nc.dram_tensor(f"bdec{i}", [128, DEC_DIMS[i + 1] // 128], F32,
                             kind="ExternalInput") for i in range(3)]
    bdec3_d = nc.dram_tensor("bdec3", [1, DEC_DIMS[4]], F32, kind="ExternalInput")
    cbaug_d = [nc.dram_tensor(f"cbaug{q}", [E_DIM + 1, N_E], F32,
                              kind="ExternalInput") for q in range(N_Q)]
    cbnat_d = [nc.dram_tensor(f"cbnat{q}", [N_E, E_DIM], F32,
                              kind="ExternalInput") for q in range(N_Q)]
    cbsqb_d = [nc.dram_tensor(f"cbsqb{q}", [128, N_E], F32,
                              kind="ExternalInput") for q in range(N_Q)]
    ident_d = nc.dram_tensor("ident", [128, 128], F32, kind="ExternalInput")
    iota_d = nc.dram_tensor("iota", [128, N_E], F32, kind="ExternalInput")
    iotaT_d = nc.dram_tensor("iotaT", [128, 2], F32, kind="ExternalInput")
    ones32_d = nc.dram_tensor("ones32", [E_DIM, 1], F32, kind="ExternalInput")
    ones1_d = nc.dram_tensor("ones1", [1, 128], F32, kind="ExternalInput")

    out_d = nc.dram_tensor("out", [BC, IN_DIM], F32, kind="ExternalOutput")
    logits_d = nc.dram_tensor("logits", [BC, N_Q, N_E], F32, kind="ExternalOutput")
    oh_d = nc.dram_tensor("oh", [BC, N_Q, N_E], F32, kind="ExternalOutput")
    idx_d = nc.dram_tensor("idx", [BC, N_Q], U32, kind="ExternalOutput")
    loss_d = nc.dram_tensor("lossp", [1, 1], F32, kind="ExternalOutput")

    ENC_K = [d // 128 for d in ENC_DIMS[:-1]]          # [8, 4, 2, 1]

    with tile.TileContext(nc) as tc:
        with tc.tile_pool(name="consts", bufs=1) as cp, \
             tc.tile_pool(name="work", bufs=1) as wp, \
             tc.tile_pool(name="mps", bufs=2, space="PSUM") as mps, \
             tc.tile_pool(name="dps", bufs=2, space="PSUM") as dps, \
             tc.tile_pool(name="sps", bufs=2, space="PSUM") as sps:

            # ---------------- consts ----------------
            ident = cp.tile([128, 128], F32)
            nc.sync.dma_start(ident[:], ident_d[:])
            iota256 = cp.tile([128, N_E], F32)
            nc.sync.dma_start(iota256[:], iota_d[:])
            iotaT = cp.tile([128, 2], F32)
            nc.sync.dma_start(iotaT[:], iotaT_d[:])
            ones32 = cp.tile([E_DIM, 1], F32)
            nc.sync.dma_start(ones32[:], ones32_d[:])
            ones1 = cp.tile([1, 128], F32)
            nc.sync.dma_start(ones1[:], ones1_d[:])

            wenc = []   # per layer: list of k-chunk tiles [128, out]
            for i in range(4):
                chunks = []
                for k in range(ENC_K[i]):
                    t = cp.tile([128, ENC_DIMS[i + 1]], F32, name=f"wenc{i}_{k}")
                    nc.sync.dma_start(t[:], wenc_d[i][k * 128:(k + 1) * 128, :])
                    chunks.append(t)
                wenc.append(chunks)
            benc = []
            for i in range(4):
                p = min(ENC_DIMS[i + 1], 128)
                t = cp.tile([p, max(ENC_DIMS[i + 1] // 128, 1)], F32, name=f"benc{i}")
                nc.sync.dma_start(t[:], benc_d[i][:])
                benc.append(t)
            wdec = []
            for i in range(3):
                chunks = []
                for k in range(max(DEC_DIMS[i] // 128, 1)):
                    p = min(DEC_DIMS[i], 128)
                    t = cp.tile([p, DEC_DIMS[i + 1]], F32, name=f"wdec{i}_{k}")
                    nc.sync.dma_start(t[:], wdec_d[i][k * 128:k * 128 + p, :])
                    chunks.append(t)
                wdec.append(chunks)
            wd3 = []
            for k in range(4):
                t = cp.tile([128, 1024], F32, name=f"wdec3_{k}")
                nc.sync.dma_start(t[:], wdec_d[3][k * 128:(k + 1) * 128, :])
                wd3.append(t)
            bdec = []
            for i in range(3):
                t = cp.tile([128, DEC_DIMS[i + 1] // 128], F32, name=f"bdec{i}")
                nc.sync.dma_start(t[:], bdec_d[i][:])
                bdec.append(t)
            bdec3 = cp.tile([1, 1024], F32)
            nc.sync.dma_start(bdec3[:], bdec3_d[:])
            cbaug, cbnat, cbsqb = [], [], []
            for q in range(N_Q):
                t = cp.tile([E_DIM + 1, N_E], F32, name=f"cbaug{q}")
                nc.sync.dma_start(t[:], cbaug_d[q][:])
                cbaug.append(t)
                c0 = cp.tile([128, E_DIM], F32, name=f"cbnat{q}_0")
                nc.sync.dma_start(c0[:], cbnat_d[q][0:128, :])
                c1 = cp.tile([128, E_DIM], F32, name=f"cbnat{q}_1")
                nc.sync.dma_start(c1[:], cbnat_d[q][128:256, :])
                cbnat.append((c0, c1))
                t = cp.tile([128, N_E], F32, name=f"cbsqb{q}")
                nc.sync.dma_start(t[:], cbsqb_d[q][:])
                cbsqb.append(t)

            lossacc = cp.tile([E_DIM, 1], F32)
            nc.vector.memset(lossacc[:], 0.0)

            # ---------------- main loop over batch tiles ----------------
            for t_i in range(NT):
                b0 = t_i * NB

                # -- load xT chunks --
                xk = []
                for k in range(8):
                    xt = wp.tile([128, NB], F32, tag="xk", bufs=10, name=f"xk{k}")
                    nc.sync.dma_start(xt[:], xT_d[k * 128:(k + 1) * 128, b0:b0 + NB])
                    xk.append(xt)

                # -- encoder (transposed-activation chain) --
                acts = xk
                for li in range(3):
                    nxt = []
                    m_chunks = ENC_DIMS[li + 1] // 128
                    for m in range(m_chunks):
                        ps = mps.tile([128, NB], F32, tag="mps", name=f"eps{li}_{m}")
                        for k in range(ENC_K[li]):
                            nc.tensor.matmul(ps[:], wenc[li][k][:, m * 128:(m + 1) * 128],
                                             acts[k][:], start=(k == 0),
                                             stop=(k == ENC_K[li] - 1))
                        a = wp.tile([128, NB], F32, tag=f"a{li}", bufs=(5 if li == 0 else 3),
                                    name=f"a{li}_{m}")
                        nc.scalar.activation(a[:], ps[:], AF.Relu,
                                             bias=benc[li][:, m:m + 1])
                        nxt.append(a)
                    acts = nxt
                # L4: latent [32, NB]
                lat_ps = sps.tile([E_DIM, NB], F32, tag="xq32", name="lat_ps")
                nc.tensor.matmul(lat_ps[:], wenc[3][0][:], acts[0][:],
                                 start=True, stop=True)
                latx = wp.tile([E_DIM + 1, NB], F32, tag="latx", bufs=2, name="latx")
                nc.scalar.activation(latx[0:E_DIM, :], lat_ps[:], AF.Identity,
                                     bias=benc[3][:, 0:1])

                # rsq for stage 0
                rT2 = wp.tile([E_DIM, NB], F32, tag="rT2", bufs=4, name="rT2_0")
                nc.vector.tensor_mul(rT2[:], latx[0:E_DIM, :], latx[0:E_DIM, :])
                rq_ps = sps.tile([1, NB], F32, tag="row1", name="rq0")
                nc.tensor.matmul(rq_ps[:], ones32[:], rT2[:], start=True, stop=True)
                nc.vector.tensor_copy(latx[E_DIM:E_DIM + 1, :], rq_ps[:])

                # -- VQ stages --
                rx = latx
                idx3 = [wp.tile([128, N_Q], U32, tag="idx3", bufs=8, name=f"idx3_{j}")
                        for j in range(NSUB)]
                r3 = None
                for s in range(N_Q):
                    idxFs = []
                    for j in range(NSUB):
                        dn = dps.tile([128, N_E], F32, tag="dn", name=f"dn{s}_{j}")
                        nc.tensor.matmul(dn[:], rx[:, j * 128:(j + 1) * 128],
                                         cbaug[s][:], start=True, stop=True)
                        dsb = wp.tile([128, N_E], F32, tag="dsb", bufs=6,
                                      name=f"dsb{s}_{j}")
                        nc.vector.tensor_tensor(dsb[:], cbsqb[s][:], dn[:],
                                                op=OP.subtract)
                        nc.sync.dma_start(
                            logits_d[b0 + j * 128:b0 + (j + 1) * 128, s, :], dsb[:])
                        dmin = wp.tile([128, 1], F32, tag="dmin", bufs=6,
                                       name=f"dmin{s}_{j}")
                        nc.vector.tensor_reduce(dmin[:], dsb[:], axis=AX.X, op=OP.min)
                        idx8 = wp.tile([128, 8], U32, tag="idx8", bufs=6,
                                       name=f"idx8{s}_{j}")
                        nc.vector.max_index(idx8[:], dmin[:].to_broadcast([128, 8]),
                                            dsb[:])
                        idxF = wp.tile([128, 1], F32, tag="idxF", bufs=6,
                                       name=f"idxF{s}_{j}")
                        nc.vector.tensor_copy(idxF[:], idx8[:, 0:1])
                        idxFs.append(idxF)
                        ohn = wp.tile([128, N_E], F32, tag="ohn", bufs=6,
                                      name=f"ohn{s}_{j}")
                        nc.gpsimd.tensor_scalar(ohn[:], iota256[:], idxF[:], None,
                                                op0=OP.is_equal)
                        nc.sync.dma_start(
                            oh_d[b0 + j * 128:b0 + (j + 1) * 128, s, :], ohn[:])
                        nc.gpsimd.tensor_copy(idx3[j][:, s:s + 1], idx8[:, 0:1])

                    # idx row [1, NB] via PE transpose, then broadcast to [128, NB]
                    ir_ps = sps.tile([1, NB], F32, tag="row1", name=f"ir{s}")
                    for j in range(NSUB):
                        nc.tensor.matmul(ir_ps[0:1, j * 128:(j + 1) * 128],
                                         idxFs[j][:], ident[:], start=True, stop=True)
                    irow = wp.tile([1, NB], F32, tag="irow", bufs=2, name=f"irow{s}")
                    nc.scalar.activation(irow[:], ir_ps[:], AF.Copy)
                    ib_ps = mps.tile([128, NB], F32, tag="mps", name=f"ib{s}")
                    nc.tensor.matmul(ib_ps[:], ones1[:], irow[:], start=True, stop=True)

                    # transposed one-hot + xq gather
                    xq_ps = sps.tile([E_DIM, NB], F32, tag="xq32", name=f"xq{s}")
                    for c in range(2):
                        ohT = wp.tile([128, NB], F32, tag="ohT", bufs=3,
                                      name=f"ohT{s}_{c}")
                        nc.vector.tensor_scalar(ohT[:], ib_ps[:], iotaT[:, c:c + 1],
                                                None, op0=OP.is_equal)
                        nc.tensor.matmul(xq_ps[:], cbnat[s][c][:], ohT[:],
                                         start=(c == 0), stop=(c == 1))

                    # residual, loss partial, next-stage rsq
                    if s < N_Q - 1:
                        rnx = wp.tile([E_DIM + 1, NB], F32, tag=f"r{s + 1}x", bufs=2,
                                      name=f"r{s + 1}x")
                        rows = rnx[0:E_DIM, :]
                    else:
                        r3 = wp.tile([E_DIM, NB], F32, tag="r3", bufs=2, name="r3")
                        rnx, rows = None, r3[:]
                    nc.vector.tensor_sub(rows, rx[0:E_DIM, :], xq_ps[:])
                    rT2n = wp.tile([E_DIM, NB], F32, tag="rT2", bufs=4,
                                   name=f"rT2_{s + 1}")
                    lp = wp.tile([E_DIM, 1], F32, tag="lp", bufs=4, name=f"lp{s}")
                    nc.scalar.activation(rT2n[:], rows, AF.Square, accum_out=lp[:])
                    nc.vector.tensor_add(lossacc[:], lossacc[:], lp[:])
                    if s < N_Q - 1:
                        rq2 = sps.tile([1, NB], F32, tag="row1", name=f"rq{s + 1}")
                        nc.tensor.matmul(rq2[:], ones32[:], rT2n[:],
                                         start=True, stop=True)
                        nc.vector.tensor_copy(rnx[E_DIM:E_DIM + 1, :], rq2[:])
                        rx = rnx

                for j in range(NSUB):
                    nc.sync.dma_start(idx_d[b0 + j * 128:b0 + (j + 1) * 128, :],
                                      idx3[j][:])

                # -- decoder --
                dec_in = wp.tile([E_DIM, NB], F32, tag="dec_in", bufs=2, name="dec_in")
                nc.vector.tensor_sub(dec_in[:], latx[0:E_DIM, :], r3[:])

                dacts = [dec_in]
                for li in range(3):
                    nxt = []
                    k_chunks = max(DEC_DIMS[li] // 128, 1)
                    for m in range(DEC_DIMS[li + 1] // 128):
                        ps = mps.tile([128, NB], F32, tag="mps", name=f"dps{li}_{m}")
                        for k in range(k_chunks):
                            nc.tensor.matmul(ps[:],
                                             wdec[li][k][:, m * 128:(m + 1) * 128],
                                             dacts[k][:], start=(k == 0),
                                             stop=(k == k_chunks - 1))
                        a = wp.tile([128, NB], F32, tag=f"da{li}",
                                    bufs=(3 if li < 2 else 5), name=f"da{li}_{m}")
                        nc.scalar.activation(a[:], ps[:], AF.Relu,
                                             bias=bdec[li][:, m:m + 1])
                        nxt.append(a)
                    dacts = nxt

                # L4 flipped: out[b, f] with act3T slices as stationary lhsT
                for j in range(NSUB):
                    for n in range(2):
                        ps = mps.tile([128, 512], F32, tag="mps", name=f"ops{j}_{n}")
                        for k in range(4):
                            nc.tensor.matmul(
                                ps[:], dacts[k][:, j * 128:(j + 1) * 128],
                                wd3[k][:, n * 512:(n + 1) * 512],
                                start=(k == 0),
                                stop=(k == 3 and not dec_b3_nonzero))
                        if dec_b3_nonzero:
                            nc.tensor.matmul(ps[:], ones1[:],
                                             bdec3[:, n * 512:(n + 1) * 512],
                                             start=False, stop=True)
                        osb = wp.tile([128, 512], F32, tag="osb", bufs=6,
                                      name=f"osb{j}_{n}")
                        nc.scalar.activation(osb[:], ps[:], AF.Copy)
                        nc.sync.dma_start(
                            out_d[b0 + j * 128:b0 + (j + 1) * 128,
                                  n * 512:(n + 1) * 512], osb[:])

            # ---------------- loss partial ----------------
            l_ps = sps.tile([1, 1], F32, tag="row1", name="l_ps")
            nc.tensor.matmul(l_ps[:], ones32[:], lossacc[:], start=True, stop=True)
            l_sb = wp.tile([1, 1], F32, tag="l_sb", bufs=1, name="l_sb")
            nc.scalar.activation(l_sb[:], l_ps[:], AF.Copy)
            nc.sync.dma_start(loss_d[:], l_sb[:])

    nc.finalize()
    return nc


def _consts(inputs):
    """Host-side packed constants (replicated across cores)."""
    f32 = np.float32
    c = {}
    for i in range(4):
        w = np.asarray(inputs[f"enc_w{i}"], f32)          # [out, in]
        c[f"wenc{i}"] = np.ascontiguousarray(w.T)          # [in, out]
        b = np.asarray(inputs[f"enc_b{i}"], f32)
        p = min(b.shape[0], 128)
        c[f"benc{i}"] = np.ascontiguousarray(b.reshape(-1, p).T)
        w = np.asarray(inputs[f"dec_w{i}"], f32)
        c[f"wdec{i}"] = np.ascontiguousarray(w.T)
        b = np.asarray(inputs[f"dec_b{i}"], f32)
        if i < 3:
            c[f"bdec{i}"] = np.ascontiguousarray(b.reshape(-1, 128).T)
    c["bdec3"] = np.asarray(inputs["dec_b3"], f32).reshape(1, -1)
    for q in range(N_Q):
        cb = np.asarray(inputs[f"cb{q}"], f32)             # [256, 32]
        c[f"cbnat{q}"] = np.ascontiguousarray(cb)
        c[f"cbaug{q}"] = np.ascontiguousarray(np.concatenate(
            [2.0 * cb.T, -np.ones((1, N_E), f32)], axis=0))
        c[f"cbsqb{q}"] = np.ascontiguousarray(np.broadcast_to(
            np.sum(cb * cb, axis=1, dtype=f32)[None, :], (128, N_E)))
    c["ident"] = np.eye(128, dtype=f32)
    c["iota"] = np.ascontiguousarray(np.broadcast_to(
        np.arange(N_E, dtype=f32)[None, :], (128, N_E)))
    c["iotaT"] = np.ascontiguousarray(
        np.arange(128, dtype=f32)[:, None] + 128.0 * np.arange(2, dtype=f32)[None, :])
    c["ones32"] = np.ones((E_DIM, 1), f32)
    c["ones1"] = np.ones((1, 128), f32)
    return c


def kernel(**inputs):
    from concourse.bass_utils import run_bass_kernel_spmd

    dec_b3 = np.asarray(inputs["dec_b3"], np.float32)
    key = bool(np.any(dec_b3))
    if key not in _CACHE:
        _CACHE[key] = _build(key)
    nc = _CACHE[key]

    consts = _consts(inputs)
    x = np.asarray(inputs["x"], np.float32)
    xT = np.ascontiguousarray(x.T)                         # [1024, 32768]
    in_maps = []
    for c_i in range(N_CORES):
        m = dict(consts)
        m["xT"] = np.ascontiguousarray(xT[:, c_i * BC:(c_i + 1) * BC])
        in_maps.append(m)

    res = run_bass_kernel_spmd(nc, in_maps, list(range(N_CORES)))
    outs = res.results

    out = np.concatenate([o["out"] for o in outs], axis=0)
    logits = np.concatenate([o["logits"] for o in outs], axis=0)
    one_hots = np.concatenate([o["oh"] for o in outs], axis=0)
    indices = np.concatenate([o["idx"] for o in outs], axis=0).astype(np.int32)
    loss_sum = float(sum(o["lossp"][0, 0] for o in outs))
    rq_loss = np.float32((1.0 + BETA) * loss_sum / (N_Q * B * E_DIM))
    return out, rq_loss, indices, one_hots, logits
